# revision 15
# baseline (speedup 1.0000x reference)
"""Deformable Conv2D (DCNv2-style) on 8 Trainium2 NeuronCores.

Strategy (data-parallel over batch, one sample per core):
  conv-first reformulation:  out[f,j] = sum_kk sum_corner w_corner[kk,j] * Y_kk[f, p_corner(kk,j)]
  where Y_kk = W[:,:,kk] @ x  (plain matmul over all spatial positions).

  Sampling uses a per-tap DRAM table TC[kk] whose row t packs the 4 bilinear
  corner pixel-vectors [Y(t-1) | Y(t) | Y(t+63) | Y(t+64)] (bf16, 1 KB), so a
  single dma_gather descriptor per (tap, output position) fetches all four
  corners.  The table is produced directly by the TensorEngine: for each
  128-row tile and each of the 4 column slots, a matmul with a shifted x tile
  as the stationary operand emits Y^T rows already in table layout (the 4x
  matmul redundancy is cheap; descriptor count and DMA bytes are not).

  Bilinear/mask/validity weights are folded into 4 per-position corner
  weights on the host (offset/mask preprocessing is data-independent of x)
  and applied as per-partition scalars via a bf16 scalar_tensor_tensor chain,
  accumulated across taps in f32.

Shapes (hardcoded per spec): x (8,128,64,64) f32, offset (8,18,64,64),
mask (8,9,64,64), weight (128,128,3,3), out (8,128,64,64) f32.
"""

import numpy as np
import ml_dtypes
from contextlib import ExitStack

import concourse.bass as bass
import concourse.bacc as bacc
import concourse.tile as tile
from concourse import mybir
from concourse.bass_utils import run_bass_kernel_spmd

B, C, H, W = 8, 128, 64, 64
F = 128
KH = KW = 3
KK = KH * KW
HW = H * W  # 4096
NP = 128
NJB = HW // NP  # 32 j-blocks
NTT = 33  # table row tiles (t in [0, 4224)); gather uses rows [0, 4160]
TROWS = NTT * NP
TCOLS = 4 * F  # 512
# column-slot source shifts: TC[t] = [Y(t-65) | Y(t-64) | Y(t-1) | Y(t)]
SLOT_SHIFT = (-65, -64, -1, 0)
XPAD_LO = 65  # x padding so shifted tiles never index out of range
XPAD = XPAD_LO + TROWS + 64  # padded x columns

BF16 = mybir.dt.bfloat16
F32 = mybir.dt.float32
I16 = mybir.dt.int16


def _prep_indices_weights(offset, mask):
    """Per-sample host prep. offset [18,H,W], mask [9,H,W] ->
    idx int16 [128, KK*256], wts f32 [128, KK*4*32]."""
    off = offset.reshape(KK, 2, H, W)
    dy, dx = off[:, 0], off[:, 1]
    ki, kj = np.meshgrid(np.arange(KH), np.arange(KW), indexing="ij")
    ki = ki.reshape(KK, 1, 1).astype(np.float32)
    kj = kj.reshape(KK, 1, 1).astype(np.float32)
    base_y = (np.arange(H, dtype=np.float32) - 1.0)[None, :, None] + ki
    base_x = (np.arange(W, dtype=np.float32) - 1.0)[None, None, :] + kj
    py = base_y + dy
    px = base_x + dx
    y0 = np.floor(py)
    x0 = np.floor(px)
    ly = (py - y0).astype(np.float32)
    lx = (px - x0).astype(np.float32)
    hy = 1.0 - ly
    hx = 1.0 - lx
    y0i = y0.astype(np.int64)
    x0i = x0.astype(np.int64)

    vy0 = (y0i >= 0) & (y0i < H)
    vy1 = (y0i + 1 >= 0) & (y0i + 1 < H)
    vx0 = (x0i >= 0) & (x0i < W)
    vx1 = (x0i + 1 >= 0) & (x0i + 1 < W)

    m = mask.reshape(KK, H, W)
    w00 = (hy * hx * m * (vy0 & vx0)).reshape(KK, HW).astype(np.float32)
    w01 = (hy * lx * m * (vy0 & vx1)).reshape(KK, HW).astype(np.float32)
    w10 = (ly * hx * m * (vy1 & vx0)).reshape(KK, HW).astype(np.float32)
    w11 = (ly * lx * m * (vy1 & vx1)).reshape(KK, HW).astype(np.float32)

    flat = np.clip(y0i * W + x0i + 65, 0, HW + 64).reshape(KK, HW)

    # idx: per kk, 4096 ordinals j wrapped o -> [o%16, o//16], replicated to
    # 128 partitions (dma_gather consumes idxs from each 16-partition group).
    idx_dev = np.empty((128, KK * 256), np.int16)
    for kk in range(KK):
        wrapped = flat[kk].astype(np.int16).reshape(256, 16).T  # [16, 256]
        idx_dev[:, kk * 256 : (kk + 1) * 256] = np.tile(wrapped, (8, 1))

    # wts: [128, (kk, corner, i)]; value[p] = w_c[kk, i*128+p]
    wts_dev = np.empty((128, KK * 4 * NJB), ml_dtypes.bfloat16)
    corners = (w00, w01, w10, w11)
    col = 0
    for kk in range(KK):
        for ci in range(4):
            wc = corners[ci][kk].reshape(NJB, 128)
            wts_dev[:, col : col + NJB] = wc.T
            col += NJB
    # wtsf: same layout as wts but f32 (ACT scale must be f32)
    wtsf_dev = np.empty((128, KK * 4 * NJB), np.float32)
    col = 0
    for kk in range(KK):
        for ci in range(4):
            wtsf_dev[:, col : col + NJB] = corners[ci][kk].reshape(NJB, 128).T
            col += NJB
    return idx_dev, wts_dev, wtsf_dev


def _split_overfull_waits(nc):
    """This walrus build accepts 1 sync-wait per instruction (2 for EVSEM).
    Move extras onto preceding same-engine NoOps."""
    for f in nc.m.functions:
        for bb in f.blocks:
            new_list = []
            for ins in bb.instructions:
                si = ins.sync_info
                waits = list(si.on_wait) if si and si.on_wait else []
                cap = 2 if isinstance(ins, mybir.InstEventSemaphore) else 1
                if len(waits) > cap:
                    extra, keep = waits[:-cap], waits[-cap:]
                    for k, w in enumerate(extra):
                        nop = mybir.InstNoOp(
                            name=f"{ins.name}_waitsplit{k}",
                            sync_info=mybir.SyncInfo(on_wait=[w], on_update=[]),
                            bass_nofuse=True,
                            engine=ins.engine,
                        )
                        new_list.append(nop)
                        nc.register_instruction(nop, overwrite=True)
                    si.on_wait = keep
                new_list.append(ins)
            bb.instructions[:] = new_list


def _build_nc():
    nc = bacc.Bacc(None, target_bir_lowering=False, debug=False)
    x_d = nc.dram_tensor("x", [NP, XPAD], BF16, kind="ExternalInput")
    wt_d = nc.dram_tensor("wt", [NP, KK * F], BF16, kind="ExternalInput")
    idx_d = nc.dram_tensor("idx", [NP, KK * 256], I16, kind="ExternalInput")
    wts_d = nc.dram_tensor("wts", [NP, KK * 4 * NJB], BF16, kind="ExternalInput")
    wtsf_d = nc.dram_tensor("wtsf", [NP, KK * 4 * NJB], F32, kind="ExternalInput")
    ident_d = nc.dram_tensor("ident", [NP, NP], F32, kind="ExternalInput")
    out_d = nc.dram_tensor("out", [NP, HW], F32, kind="ExternalOutput")
    tbl_d = nc.dram_tensor("tbl", [KK, TROWS, TCOLS], BF16, kind="Internal")

    TBL_KK = TROWS * TCOLS

    with tile.TileContext(nc) as tc, ExitStack() as ctx:
        cpool = ctx.enter_context(tc.tile_pool(name="const", bufs=1))
        tcst_pool = ctx.enter_context(tc.tile_pool(name="tcst", bufs=2))
        gpool = ctx.enter_context(tc.tile_pool(name="gat", bufs=3))
        tpool = ctx.enter_context(tc.tile_pool(name="tmp", bufs=6))
        accpool = ctx.enter_context(tc.tile_pool(name="acc", bufs=1))
        opool = ctx.enter_context(tc.tile_pool(name="ot", bufs=4))
        pspool = ctx.enter_context(tc.tile_pool(name="ps", bufs=2, space="PSUM"))

        x_sb = cpool.tile([NP, XPAD], BF16)
        wt_sb = cpool.tile([NP, KK * F], BF16)
        idx_sb = cpool.tile([NP, KK * 256], I16)
        wts_sb = cpool.tile([NP, KK * 4 * NJB], BF16)
        wtsf_sb = cpool.tile([NP, KK * 4 * NJB], F32)
        id_sb = cpool.tile([NP, NP], F32)
        acc_sb = accpool.tile([NP, HW], F32)
        out_sb = accpool.tile([NP, HW], F32)

        nc.sync.dma_start(x_sb[:], x_d[:])
        nc.sync.dma_start(wt_sb[:], wt_d[:])
        nc.sync.dma_start(idx_sb[:], idx_d[:])
        nc.sync.dma_start(wts_sb[:], wts_d[:])
        nc.sync.dma_start(wtsf_sb[:], wtsf_d[:])
        nc.sync.dma_start(id_sb[:], ident_d[:])

        # ---- Stage A: build TC tables.
        # per (tt, slot): stationary = shifted x tile; 3 matmuls (groups of
        # 3 kk, N=384) -> psum f32 -> evict (cast bf16) into tcst staging
        # [q, (kk, slot, f)]; one DMA per tt writes 9 kk tables' rows.
        for g in range(3):
            for tt in range(NTT):
                tcst = tcst_pool.tile([NP, 3, 4, F], BF16)
                ps = pspool.tile([NP, 4, 512], F32)
                for s in range(4):
                    xoff = XPAD_LO + tt * NP + SLOT_SHIFT[s]
                    nc.tensor.matmul(
                        ps[:, s, 0 : 3 * F],
                        x_sb[:, xoff : xoff + NP],
                        wt_sb[:, g * 3 * F : (g + 1) * 3 * F],
                        start=True,
                        stop=True,
                    )
                # one eviction: psum [p, s, (kk_local, f)] -> tcst [p, kk_local, s, f]
                src_ap = ps[:, :, 0 : 3 * F].rearrange("p s (k f) -> p s k f", k=3)
                nc.scalar.copy(tcst[:].rearrange("p k s f -> p s k f"), src_ap)
                dst = bass.AP(
                    tbl_d,
                    3 * g * TBL_KK + tt * NP * TCOLS,
                    [[TCOLS, NP], [TBL_KK, 3], [1, TCOLS]],
                )
                nc.sync.dma_start(dst, tcst[:])

        # ---- Stage B: gather + weighted accumulate
        for kk in range(KK):
            g_t = gpool.tile([NP, NJB, TCOLS], BF16)
            src = bass.AP(tbl_d, kk * TBL_KK, [[TCOLS, HW + 65], [1, TCOLS]])
            nc.gpsimd.dma_gather(
                out_ap=g_t[:],
                in_ap=src,
                idxs_ap=idx_sb[:, kk * 256 : (kk + 1) * 256],
                num_idxs=HW,
                num_idxs_reg=HW,
                elem_size=TCOLS,
                single_packet=False,
            )
            wbase = kk * 4 * NJB
            for i in range(NJB):
                a_sl = acc_sb[:, i * NP : (i + 1) * NP]
                w0 = wtsf_sb[:, wbase + i : wbase + i + 1]
                w1 = wts_sb[:, wbase + NJB + i : wbase + NJB + i + 1]
                w2 = wts_sb[:, wbase + 2 * NJB + i : wbase + 2 * NJB + i + 1]
                w3 = wts_sb[:, wbase + 3 * NJB + i : wbase + 3 * NJB + i + 1]
                t_b = tpool.tile([NP, NP], BF16, tag="tb")
                nc.scalar.mul(t_b[:], g_t[:, i, 0:F], w0)
                nc.vector.scalar_tensor_tensor(
                    t_b[:], g_t[:, i, F : 2 * F], w1, t_b[:],
                    mybir.AluOpType.mult, mybir.AluOpType.add,
                )
                nc.vector.scalar_tensor_tensor(
                    t_b[:], g_t[:, i, 2 * F : 3 * F], w2, t_b[:],
                    mybir.AluOpType.mult, mybir.AluOpType.add,
                )
                if kk == 0:
                    nc.vector.scalar_tensor_tensor(
                        a_sl, g_t[:, i, 3 * F : 4 * F], w3, t_b[:],
                        mybir.AluOpType.mult, mybir.AluOpType.add,
                    )
                else:
                    nc.vector.scalar_tensor_tensor(
                        t_b[:], g_t[:, i, 3 * F : 4 * F], w3, t_b[:],
                        mybir.AluOpType.mult, mybir.AluOpType.add,
                    )
                    nc.vector.tensor_add(a_sl, a_sl, t_b[:])

        # ---- Stage C: transpose acc [p, f] tiles -> out [f, j]
        for jb in range(NJB):
            pst = pspool.tile([NP, NP], F32, tag="ps")
            nc.tensor.transpose(pst[:], acc_sb[:, jb * NP : (jb + 1) * NP], id_sb[:])
            if jb % 2 == 0:
                nc.scalar.copy(out_sb[:, jb * NP : (jb + 1) * NP], pst[:])
            else:
                nc.vector.tensor_copy(out_sb[:, jb * NP : (jb + 1) * NP], pst[:])
        nc.sync.dma_start(out_d[:], out_sb[:])

    nc.compile()
    _split_overfull_waits(nc)
    return nc


_NC_CACHE = {}


def _get_nc():
    if "nc" not in _NC_CACHE:
        _NC_CACHE["nc"] = _build_nc()
    return _NC_CACHE["nc"]


def _prep_x(xb):
    """x [C,H,W] f32 -> padded bf16 [128, XPAD]."""
    xp = np.zeros((C, XPAD), ml_dtypes.bfloat16)
    xp[:, XPAD_LO : XPAD_LO + HW] = xb.reshape(C, HW).astype(ml_dtypes.bfloat16)
    return xp


def kernel(x, offset, mask, weight, **run_kwargs):
    x = np.asarray(x, np.float32)
    offset = np.asarray(offset, np.float32)
    mask = np.asarray(mask, np.float32)
    weight = np.asarray(weight, np.float32)

    wt = np.transpose(weight.reshape(F, C, KK), (1, 2, 0)).reshape(C, KK * F)
    wt = np.ascontiguousarray(wt).astype(ml_dtypes.bfloat16)
    ident = np.eye(NP, dtype=np.float32)

    in_maps = []
    for b in range(B):
        idx_dev, wts_dev, wtsf_dev = _prep_indices_weights(offset[b], mask[b])
        in_maps.append(
            {
                "x": _prep_x(x[b]),
                "wt": wt,
                "idx": idx_dev,
                "wts": wts_dev,
                "wtsf": wtsf_dev,
                "ident": ident,
            }
        )

    nc = _get_nc()
    res = run_bass_kernel_spmd(nc, in_maps, core_ids=list(range(8)), **run_kwargs)
    out = np.stack([np.asarray(res.results[b]["out"]).reshape(F, H, W) for b in range(B)])
    if run_kwargs:
        kernel.last_results = res
    return out
